# revision 8
# baseline (speedup 1.0000x reference)
"""Trainium2 Bass kernel for additive-attention pooling.

reference math:
    scores[b,t] = tanh(q[b]) @ vw_a + tanh(c[b,t]) @ vw_b
    attn        = softmax(where(mask<1, -1e10, scores), axis=t)
    out[b,e]    = sum_t attn[b,t] * c[b,t,e]

Softmax is shift-invariant and the query term is constant over t, so the
output does not depend on `query` or `v_w[:E]` at all.  Per batch row,
in a single pass over context:
    s_t  = sum_e (tanh(c_te) + mbias_t) * w2_e
         = tanh(c_t).w2 + (mask_t-1)*1e9     (DVE affine_mul_reduce, f32;
                                              per-partition bias (m-1)*1e9/S,
                                              S = sum(w2), pushes masked rows
                                              to score ~ -1e9)
    p_t  = exp(s_t)                          (ACT, bf16 out; masked -> 0)
    out  = (sum_t p_t*c_t) / (sum_t p_t)     (PE bf16 matmuls, f32 PSUM)

Engine placement (measured): f32 matmul runs 2-pass LOW_HIGH on PE (~4x
bf16 cost), and every engine's explicit f32->bf16 cast is too slow
(GPSIMD 3.7us, DVE/ACT ~1.1us per tile) — so the matmul rhs is a ZERO
COST bf16 view of the f32 tile: bitcast to bf16 and read the odd
(high-half) 2-byte lanes with stride 2.  That is exactly bf16
truncation of each f32 (~1ulp, fine for the 2e-2 gate).  A f32 1.0
ones-column embedded in each tile (bf16-view 1.0 exactly) makes the
same matmul accumulate the softmax denominator.

t-tiles pack 2 context rows per partition ([128 x (j=2, 769)]) so each
partition streams 6KB contiguous from HBM and tanh batches to one ACT
op per 256 rows.

Sharding: pure data parallel, batch 16 -> 2 per core on 8 cores; w2
replicated.  No collectives needed.
"""

import sys

for _p in ("/opt/trn_rl_repo", "/root/.axon_site/_ro/trn_rl_repo"):
    if _p not in sys.path:
        sys.path.append(_p)

import numpy as np

B, T, E = 16, 4096, 768
NCORES = 8
BPC = B // NCORES  # batches per core
P = 128            # partitions per tile
J = 4              # context rows per partition
G = T // (P * J)   # 16 t-tiles per batch
NEG_BIG = 1.0e9    # exp(-1e9) == 0.0
EB = E + 1         # tile row: 768 data + 1 ones column

_cache = {}


def _build_program():
    import concourse.tile as tile
    from concourse import bacc, mybir

    f32 = mybir.dt.float32
    bf16 = mybir.dt.bfloat16
    i32 = mybir.dt.int32
    AF = mybir.ActivationFunctionType
    ALU = mybir.AluOpType

    nc = bacc.Bacc(
        "TRN2",
        target_bir_lowering=False,
        debug=False,
        enable_asserts=False,
        num_devices=NCORES,
    )
    ctx_d = nc.dram_tensor("ctx", [BPC, T, E], f32, kind="ExternalInput")
    mask_d = nc.dram_tensor("mask", [BPC, T], i32, kind="ExternalInput")
    w2_d = nc.dram_tensor("w2", [1, E], f32, kind="ExternalInput")
    out_d = nc.dram_tensor("out", [BPC, E], f32, kind="ExternalOutput")

    with tile.TileContext(nc) as tc:
        with (
            tc.tile_pool(name="const", bufs=1) as const_pool,
            tc.tile_pool(name="cin", bufs=4) as c_pool,
            tc.tile_pool(name="tanh", bufs=3) as t_pool,
            tc.tile_pool(name="small", bufs=8) as s_pool,
            tc.tile_pool(name="batch", bufs=2) as b_pool,
            tc.tile_pool(name="paccum", bufs=2, space="PSUM") as pa_pool,
        ):
            # ---- constants ----
            w2_row = const_pool.tile([1, E], f32)
            nc.sync.dma_start(w2_row[:], w2_d[:])
            w2_rep = const_pool.tile([P, E], f32)
            nc.sync.dma_start(w2_rep[:], w2_d[:].broadcast_to([P, E]))
            # R = NEG_BIG / S with S = sum(w2): amr bias C1 = (m-1)*R gives
            # C1*S = -NEG_BIG on masked rows
            s_sum = const_pool.tile([1, 1], f32)
            nc.vector.reduce_sum(s_sum[:], w2_row[:], axis=mybir.AxisListType.X)
            r_one = const_pool.tile([1, 1], f32)
            nc.vector.reciprocal(r_one[:], s_sum[:])
            nc.scalar.mul(r_one[:], r_one[:], NEG_BIG)
            r_rep = const_pool.tile([P, 1], f32)
            nc.gpsimd.partition_broadcast(r_rep[:, :], r_one[:])

            for b in range(BPC):
                # mask -> per-(p, g*J+j) amr bias: 0 kept, -R masked
                mask_i = b_pool.tile([P, G * J], i32)
                nc.sync.dma_start(
                    mask_i[:].rearrange("p (g j) -> p g j", g=G, j=J),
                    mask_d[b].rearrange("(g p j) -> p g j", p=P, j=J),
                )
                mask_f = b_pool.tile([P, G * J], f32)
                nc.vector.tensor_copy(mask_f[:], mask_i[:])
                mbias = b_pool.tile([P, G * J], f32)
                nc.vector.tensor_scalar(
                    mbias[:], mask_f[:], r_rep[:], r_rep[:],
                    op0=ALU.mult, op1=ALU.subtract,
                )

                acc = pa_pool.tile([1, EB], f32)  # [sum p*c | sum p]

                for g in range(G):
                    c = c_pool.tile([P, J * EB], f32)
                    nc.sync.dma_start(
                        c[:].rearrange("p (j e) -> p j e", j=J)[:, :, 0:E],
                        ctx_d[b, g * P * J:(g + 1) * P * J, :].rearrange(
                            "(p j) e -> p j e", j=J
                        ),
                    )
                    # ones columns at the end of each j slice (f32 1.0 is
                    # exactly 1.0 in the truncated-bf16 view)
                    nc.vector.memset(
                        c[:].rearrange("p (j e) -> p j e", j=J)[:, :, E:EB], 1.0
                    )
                    # zero-cost truncated-bf16 view: odd u16 lane of each f32
                    c_hi = c[:].bitcast(bf16).rearrange(
                        "p (n two) -> p n two", two=2
                    )[:, :, 1]

                    th = t_pool.tile([P, J * E], f32)
                    nc.scalar.activation(
                        th[:].rearrange("p (j e) -> p j e", j=J),
                        c[:].rearrange("p (j e) -> p j e", j=J)[:, :, 0:E],
                        AF.Tanh,
                    )

                    s2 = s_pool.tile([P, J], f32)
                    for j in range(J):
                        sl = slice(j * E, (j + 1) * E)
                        nc.vector.affine_mul_reduce(
                            th[:, sl], s2[:, j:j + 1], th[:, sl], w2_rep[:],
                            1.0, mbias[:, g * J + j:g * J + j + 1],
                        )

                    p2 = s_pool.tile([P, J], bf16)
                    nc.scalar.activation(p2[:], s2[:], AF.Exp)

                    first, last = g == 0, g == G - 1
                    for j in range(J):
                        lhsT = p2[:, j:j + 1]
                        st = first and j == 0
                        sp = last and j == J - 1
                        nc.tensor.matmul(
                            acc[:, 0:512], lhsT=lhsT,
                            rhs=c_hi[:, j * EB:j * EB + 512], start=st, stop=sp,
                        )
                        nc.tensor.matmul(
                            acc[:, 512:EB], lhsT=lhsT,
                            rhs=c_hi[:, j * EB + 512:(j + 1) * EB],
                            start=st, stop=sp,
                        )

                recip = s_pool.tile([1, 1], f32)
                nc.vector.reciprocal(recip[:], acc[0:1, E:EB])
                out_sb = s_pool.tile([1, E], f32)
                nc.vector.tensor_scalar_mul(out_sb[:], acc[:, 0:E], recip[:])
                nc.sync.dma_start(out_d[b:b + 1, :], out_sb[:])

    nc.compile()
    return nc


def _get_program():
    if "nc" not in _cache:
        _cache["nc"] = _build_program()
    return _cache["nc"]


def kernel(query, context, mask, v_w):
    from concourse.bass_utils import run_bass_kernel_spmd

    nc = _get_program()
    w2 = np.ascontiguousarray(v_w[E:]).reshape(1, E).astype(np.float32)
    in_maps = [
        {
            "ctx": np.ascontiguousarray(context[i * BPC:(i + 1) * BPC]),
            "mask": np.ascontiguousarray(mask[i * BPC:(i + 1) * BPC]),
            "w2": w2,
        }
        for i in range(NCORES)
    ]
    res = run_bass_kernel_spmd(nc, in_maps, list(range(NCORES)))
    return np.concatenate([res.results[i]["out"] for i in range(NCORES)], axis=0)


# revision 16
# speedup vs baseline: 1.1458x; 1.1458x over previous
"""Trainium2 Bass kernel for additive-attention pooling.

reference math:
    scores[b,t] = tanh(q[b]) @ vw_a + tanh(c[b,t]) @ vw_b
    attn        = softmax(where(mask<1, -1e10, scores), axis=t)
    out[b,e]    = sum_t attn[b,t] * c[b,t,e]

Softmax is shift-invariant and the query term is constant over t, so the
output does not depend on `query` or `v_w[:E]` at all.  Per batch row,
in a single pass over context:
    s_t  = sum_e (tanh(c_te) + mbias_t) * w2_e
         = tanh(c_t).w2 + (mask_t-1)*1e9     (DVE affine_mul_reduce, f32;
                                              per-partition bias (m-1)*1e9/S,
                                              S = sum(w2), pushes masked rows
                                              to score ~ -1e9)
    p_t  = exp(s_t)                          (ACT, bf16 out; masked -> 0)
    out  = (sum_t p_t*c_t) / (sum_t p_t)     (PE bf16 matmuls, f32 PSUM)

Engine placement (measured): f32 matmul runs 2-pass LOW_HIGH on PE (~4x
bf16 cost), and every engine's explicit f32->bf16 cast is too slow
(GPSIMD 3.7us, DVE/ACT ~1.1us per tile) — so the matmul rhs is a ZERO
COST bf16 view of the f32 tile: bitcast to bf16 and read the odd
(high-half) 2-byte lanes with stride 2.  That is exactly bf16
truncation of each f32 (~1ulp, fine for the 2e-2 gate).  A f32 1.0
ones-column embedded in each tile (bf16-view 1.0 exactly) makes the
same matmul accumulate the softmax denominator.

t-tiles pack 2 context rows per partition ([128 x (j=2, 769)]) so each
partition streams ~6KB from HBM per tile and tanh batches to one ACT
op per 256 rows.  w2 (replicated to 128 partitions) and the mask-bias
scale R = 1e9/sum(w2) are prepared host-side — they are tiny and would
otherwise serialize ~10us of on-device setup before the first score op.

Sharding: pure data parallel, batch 16 -> 2 per core on 8 cores; w2
replicated.  No collectives needed.
"""

import sys

for _p in ("/opt/trn_rl_repo", "/root/.axon_site/_ro/trn_rl_repo"):
    if _p not in sys.path:
        sys.path.append(_p)

import numpy as np

B, T, E = 16, 4096, 768
NCORES = 8
BPC = B // NCORES  # batches per core
P = 128            # partitions per tile
J = 2              # context rows per partition
G = T // (P * J)   # 16 t-tiles per batch
NEG_BIG = 1.0e9    # exp(-1e9) == 0.0
EB = E + 1         # tile row: 768 data + 1 ones column

_cache = {}


def _build_program():
    import concourse.tile as tile
    from concourse import bacc, mybir

    f32 = mybir.dt.float32
    bf16 = mybir.dt.bfloat16
    i32 = mybir.dt.int32
    AF = mybir.ActivationFunctionType
    ALU = mybir.AluOpType

    nc = bacc.Bacc(
        "TRN2",
        target_bir_lowering=False,
        debug=False,
        enable_asserts=False,
        num_devices=NCORES,
    )
    ctx_d = nc.dram_tensor("ctx", [BPC, T, E], f32, kind="ExternalInput")
    mask_d = nc.dram_tensor("mask", [BPC, T], i32, kind="ExternalInput")
    w2_d = nc.dram_tensor("w2rep", [P, E], f32, kind="ExternalInput")
    r_d = nc.dram_tensor("rrep", [P, 1], f32, kind="ExternalInput")
    out_d = nc.dram_tensor("out", [BPC, E], f32, kind="ExternalOutput")

    with tile.TileContext(nc) as tc:
        with (
            tc.tile_pool(name="const", bufs=1) as const_pool,
            tc.tile_pool(name="cin", bufs=6) as c_pool,
            tc.tile_pool(name="tanh", bufs=4) as t_pool,
            tc.tile_pool(name="small", bufs=8) as s_pool,
            tc.tile_pool(name="batch", bufs=2) as b_pool,
            tc.tile_pool(name="paccum", bufs=2, space="PSUM") as pa_pool,
        ):
            def load_tile(b, g):
                c = c_pool.tile([P, J * EB], f32)
                nc.sync.dma_start(
                    c[:].rearrange("p (j e) -> p j e", j=J)[:, :, 0:E],
                    ctx_d[b, g * P * J:(g + 1) * P * J, :].rearrange(
                        "(p j) e -> p j e", j=J
                    ),
                )
                # ones columns at the end of each j slice (f32 1.0 is
                # exactly 1.0 in the truncated-bf16 view); GPSIMD is idle
                nc.gpsimd.memset(
                    c[:].rearrange("p (j e) -> p j e", j=J)[:, :, E:EB], 1.0
                )
                return c

            # ---- constants (prepared host-side, one DMA each) ----
            w2_rep = const_pool.tile([P, E], f32)
            nc.sync.dma_start(w2_rep[:], w2_d[:])
            r_rep = const_pool.tile([P, 1], f32)
            nc.sync.dma_start(r_rep[:], r_d[:])

            for b in range(BPC):
                # mask -> per-(p, g*J+j) amr bias: 0 kept, -R masked
                mask_i = b_pool.tile([P, G * J], i32)
                nc.sync.dma_start(
                    mask_i[:].rearrange("p (g j) -> p g j", g=G, j=J),
                    mask_d[b].rearrange("(g p j) -> p g j", p=P, j=J),
                )
                mask_f = b_pool.tile([P, G * J], f32)
                nc.vector.tensor_copy(mask_f[:], mask_i[:])
                mbias = b_pool.tile([P, G * J], f32)
                nc.vector.tensor_scalar(
                    mbias[:], mask_f[:], r_rep[:], r_rep[:],
                    op0=ALU.mult, op1=ALU.subtract,
                )

                acc = pa_pool.tile([1, EB], f32)  # [sum p*c | sum p]

                for g in range(G):
                    c = load_tile(b, g)
                    # zero-cost truncated-bf16 view: odd u16 lane of each f32
                    c_hi = c[:].bitcast(bf16).rearrange(
                        "p (n two) -> p n two", two=2
                    )[:, :, 1]

                    th = t_pool.tile([P, J * E], f32)
                    nc.scalar.activation(
                        th[:].rearrange("p (j e) -> p j e", j=J),
                        c[:].rearrange("p (j e) -> p j e", j=J)[:, :, 0:E],
                        AF.Tanh,
                    )

                    s2 = s_pool.tile([P, J], f32)
                    for j in range(J):
                        sl = slice(j * E, (j + 1) * E)
                        nc.vector.affine_mul_reduce(
                            th[:, sl], s2[:, j:j + 1], th[:, sl], w2_rep[:],
                            1.0, mbias[:, g * J + j:g * J + j + 1],
                        )

                    p2 = s_pool.tile([P, J], bf16)
                    nc.scalar.activation(p2[:], s2[:], AF.Exp)

                    first, last = g == 0, g == G - 1
                    for j in range(J):
                        lhsT = p2[:, j:j + 1]
                        st = first and j == 0
                        sp = last and j == J - 1
                        nc.tensor.matmul(
                            acc[:, 0:512], lhsT=lhsT,
                            rhs=c_hi[:, j * EB:j * EB + 512], start=st, stop=sp,
                        )
                        nc.tensor.matmul(
                            acc[:, 512:EB], lhsT=lhsT,
                            rhs=c_hi[:, j * EB + 512:(j + 1) * EB],
                            start=st, stop=sp,
                        )

                recip = s_pool.tile([1, 1], f32)
                nc.vector.reciprocal(recip[:], acc[0:1, E:EB])
                out_sb = s_pool.tile([1, E], f32)
                nc.vector.tensor_scalar_mul(out_sb[:], acc[:, 0:E], recip[:])
                nc.sync.dma_start(out_d[b:b + 1, :], out_sb[:])

    nc.compile()
    return nc


def _get_program():
    if "nc" not in _cache:
        _cache["nc"] = _build_program()
    return _cache["nc"]


def kernel(query, context, mask, v_w):
    import time
    from concourse.bass_utils import run_bass_kernel_spmd

    nc = _get_program()
    w2 = np.asarray(v_w[E:], dtype=np.float32)
    w2_rep = np.ascontiguousarray(np.broadcast_to(w2, (P, E)))
    r = np.float32(NEG_BIG) / w2.sum(dtype=np.float32)
    r_rep = np.full((P, 1), r, dtype=np.float32)
    in_maps = [
        {
            "ctx": np.ascontiguousarray(context[i * BPC:(i + 1) * BPC]),
            "mask": np.ascontiguousarray(mask[i * BPC:(i + 1) * BPC]),
            "w2rep": w2_rep,
            "rrep": r_rep,
        }
        for i in range(NCORES)
    ]
    last_err = None
    for attempt in range(3):
        try:
            res = run_bass_kernel_spmd(nc, in_maps, list(range(NCORES)))
            return np.concatenate(
                [res.results[i]["out"] for i in range(NCORES)], axis=0
            )
        except Exception as e:  # transient axon/device hiccups
            last_err = e
            time.sleep(5)
    raise last_err
